# revision 14
# baseline (speedup 1.0000x reference)
"""Trainium2 Bass kernel for nn_DGT_6485400616966 (soft decision tree forward).

Math (forward pass only): the straight-through/one-hot structure collapses to a
10-level tree descent following sign(x . W_pred[node]) at visited nodes;
out = softmax(W_or[:, leaf]); std = clip(action_stds[:, leaf], -20, 2).

Device algorithm per core (8192 samples, data-parallel over 8 cores):
  1. Dense fp32 matmul for tree levels 0..7 only (255 nodes, 256 cols):
     z = x @ W[0:255].T exactly (fp32 PE mode), ACT evicts sign bits.
  2. DVE bottom-up collapse of the 8-level subtree -> j8 [128, 64] (the
     level-8 node index per sample, sample-part layout; sample j sits at
     partition j%128, btile j//128).
  3. Levels 8 and 9 are resolved per sample: wrap j into the gpsimd idx
     layout ([16, Q] wrapped + replicated via small SBUF-SBUF DMAs), gather
     W columns with ap_gather (dim-major, tables resident in SBUF), multiply
     with the same xT tiles on DVE, reduce over the 256 dims with gpsimd
     partition_all_reduce (fp16 out), PE-transpose the replicated result
     back to sample-part, and update j9 = 2*j8 + (z8 < 0), leaf = 2*j9 + u9.
  4. Final ap_gather from class-replicated softmax/std tables by leaf,
     PE transpose, contiguous DMA out in natural sample order.
"""

import sys

for _p in ("/opt/trn_rl_repo",):
    if _p not in sys.path:
        sys.path.insert(0, _p)

from contextlib import ExitStack

import numpy as np

import concourse.bacc as bacc
import concourse.bass as bass
import concourse.bass_isa as bass_isa
import concourse.tile as tile
from concourse import mybir
from concourse.bass_utils import run_bass_kernel_spmd

HEIGHT = 10
IN_DIM = 256
OUT_DIM = 16
BATCH = 65536
N_CORES = 8
B_LOC = BATCH // N_CORES          # 8192 samples per core
NT = B_LOC // 128                 # 64 btiles of 128 samples
NB = 8                            # btiles per dense chunk
NCH = NT // NB                    # 8 dense chunks
DNODES = 256                      # dense cols: nodes 0..254 + 1 pad
GC = 2048                         # samples per gathered-stage chunk
NGC = B_LOC // GC                 # 4 gathered chunks
F32 = mybir.dt.float32
FP16 = mybir.dt.float16
I16 = mybir.dt.int16


def _build(nc, use_bias: bool):
    xT = nc.dram_tensor("xT", [IN_DIM, B_LOC], F32, kind="ExternalInput")
    Wd = nc.dram_tensor("Wd", [IN_DIM, DNODES], F32, kind="ExternalInput")
    W8 = nc.dram_tensor("W8", [IN_DIM, 256], F32, kind="ExternalInput")
    W9 = nc.dram_tensor("W9", [IN_DIM, 512], F32, kind="ExternalInput")
    Tout = nc.dram_tensor("Tout", [128, 1024], F32, kind="ExternalInput")
    Tstd = nc.dram_tensor("Tstd", [128, 1024], F32, kind="ExternalInput")
    TH = nc.dram_tensor("TH", [128, DNODES], F32, kind="ExternalInput")
    B89 = nc.dram_tensor("B89", [128, 768], F32, kind="ExternalInput")
    Ident = nc.dram_tensor("Ident", [128, 128], F32, kind="ExternalInput")
    out_o = nc.dram_tensor("out_o", [B_LOC, OUT_DIM], F32, kind="ExternalOutput")
    out_s = nc.dram_tensor("out_s", [B_LOC, OUT_DIM], F32, kind="ExternalOutput")

    with tile.TileContext(nc) as tc, ExitStack() as ctx:
        consts = ctx.enter_context(tc.tile_pool(name="consts", bufs=1))
        spool = ctx.enter_context(tc.tile_pool(name="spool", bufs=3))
        rpool = ctx.enter_context(tc.tile_pool(name="rpool", bufs=3))
        dpool = ctx.enter_context(tc.tile_pool(name="dpool", bufs=3))
        gpool = ctx.enter_context(tc.tile_pool(name="gpool", bufs=2))
        ipool = ctx.enter_context(tc.tile_pool(name="ipool", bufs=2))
        opool = ctx.enter_context(tc.tile_pool(name="opool", bufs=2))
        zpool = ctx.enter_context(
            tc.tile_pool(name="zpool", bufs=3, space=bass.MemorySpace.PSUM)
        )
        tpool = ctx.enter_context(
            tc.tile_pool(name="tpool", bufs=2, space=bass.MemorySpace.PSUM)
        )

        # persistent tiles
        x = [
            consts.tile([128, B_LOC], F32, tag=f"x{k}", name=f"x{k}")
            for k in range(2)
        ]
        wd = [
            consts.tile([128, DNODES], F32, tag=f"wd{k}", name=f"wd{k}")
            for k in range(2)
        ]
        w8 = [
            consts.tile([128, 256], F32, tag=f"w8{k}", name=f"w8{k}")
            for k in range(2)
        ]
        w9 = [
            consts.tile([128, 512], F32, tag=f"w9{k}", name=f"w9{k}")
            for k in range(2)
        ]
        t_out = consts.tile([128, 1024], F32)
        t_std = consts.tile([128, 1024], F32)
        ident = consts.tile([128, 128], F32)
        ident16 = consts.tile([128, 128], FP16)
        th = consts.tile([128, DNODES], F32) if use_bias else None
        b89 = consts.tile([128, 768], F32) if use_bias else None

        j8sp = consts.tile([128, NT], FP16)   # level-8 node idx, sample-part
        j9sp = consts.tile([128, NT], FP16)
        leafsp = consts.tile([128, NT], FP16)
        leafi = consts.tile([128, NT], I16)

        # late-loaded consts (first consumed after the first dense chunk)
        def load_late_consts():
            for k in range(2):
                ks = slice(128 * k, 128 * (k + 1))
                nc.sync.dma_start(out=wd[k], in_=Wd[ks, :])
                nc.sync.dma_start(out=w8[k], in_=W8[ks, :])
                nc.sync.dma_start(out=w9[k], in_=W9[ks, :])
            nc.sync.dma_start(out=t_out, in_=Tout[:, :])
            nc.sync.dma_start(out=t_std, in_=Tstd[:, :])
            nc.sync.dma_start(out=ident, in_=Ident[:, :])
            nc.vector.tensor_copy(out=ident16, in_=ident)
            if use_bias:
                nc.sync.dma_start(out=th, in_=TH[:, :])
                nc.sync.dma_start(out=b89, in_=B89[:, :])

        # W for dense chunk 0 must be there first
        nc.sync.dma_start(out=wd[0][:, :], in_=Wd[0:128, :])
        nc.sync.dma_start(out=wd[1][:, :], in_=Wd[128:256, :])

        o_view = out_o.rearrange(
            "(b q g r) c -> b q r g c", b=8, q=8, g=8, r=16
        )
        s_view = out_s.rearrange(
            "(b q g r) c -> b q r g c", b=8, q=8, g=8, r=16
        )

        # ---------------- dense stage: one chunk of NB btiles ----------------
        def dense_chunk(c):
            s_chunk = spool.tile([128, NB, DNODES], FP16, tag="s")
            for k in range(NB):
                t = c * NB + k
                if k == 0:
                    hs = slice(128 * NB * c, 128 * NB * (c + 1))
                    for kk in range(2):
                        ks = slice(128 * kk, 128 * (kk + 1))
                        nc.sync.dma_start(out=x[kk][:, hs], in_=xT[ks, hs])
                    if c == 0:
                        load_late_consts()
                bs = slice(128 * t, 128 * (t + 1))
                z = zpool.tile([128, DNODES], F32, tag="z", name="z")
                for kk in range(2):
                    nc.tensor.matmul(
                        z[:, :],
                        x[kk][:, bs],
                        wd[kk][:, :],
                        start=(kk == 0),
                        stop=(kk == 1),
                    )
                # u = (z < -b): ACT saturated-sigmoid trick for b == 0
                if use_bias:
                    nc.vector.tensor_tensor(
                        out=s_chunk[:, k, :],
                        in0=z[:, :],
                        in1=th[:, :],
                        op=mybir.AluOpType.is_lt,
                    )
                else:
                    nc.scalar.activation(
                        out=s_chunk[:, k, :],
                        in_=z[:, :],
                        func=mybir.ActivationFunctionType.Sigmoid,
                        scale=-1e30,
                    )

            # bottom-up collapse of the 8-level subtree (nodes 0..254)
            r_prev = s_chunk[:, :, 127:255]
            for i in range(6, -1, -1):
                n = 1 << i
                kconst = float(1 << (7 - i))
                u_i = s_chunk[:, :, n - 1 : 2 * n - 1]
                rp = r_prev.rearrange("p b (n two) -> p b n two", two=2)
                r_e = rp[:, :, :, 0]
                r_o = rp[:, :, :, 1]
                d_t = dpool.tile([128, NB, n], FP16, tag="d", name="d_t")
                nc.vector.scalar_tensor_tensor(
                    out=d_t,
                    in0=r_o,
                    scalar=kconst,
                    in1=r_e,
                    op0=mybir.AluOpType.add,
                    op1=mybir.AluOpType.subtract,
                )
                nc.vector.tensor_tensor(
                    out=d_t, in0=u_i, in1=d_t, op=mybir.AluOpType.mult
                )
                if i > 0:
                    r_t = rpool.tile([128, NB, n], FP16, tag="r", name="r_t")
                    nc.vector.tensor_tensor(
                        out=r_t, in0=r_e, in1=d_t, op=mybir.AluOpType.add
                    )
                    r_prev = r_t
                else:
                    nc.vector.tensor_tensor(
                        out=j8sp[:, c * NB : (c + 1) * NB],
                        in0=r_e[:, :, 0],
                        in1=d_t[:, :, 0],
                        op=mybir.AluOpType.add,
                    )

        # ------------- gathered-level helpers (per chunk of GC) -------------
        QW = GC // 16            # wrapped free size (128)
        NTC = GC // 128          # btiles per gathered chunk (16)

        def wrap_and_bcast(src_sp_i16, c, nm):
            # [128, NTC] sample-part -> [16, QW] wrapped -> [128, QW] replicated
            jw16 = ipool.tile([16, QW], I16, tag=f"jw16{nm}", name="jw16")
            wv = jw16.rearrange("r (b k) -> r k b", k=8)
            for k in range(8):
                nc.sync.dma_start(
                    out=wv[:, k, :],
                    in_=src_sp_i16[16 * k : 16 * k + 16, :],
                )
            jw128 = ipool.tile([128, QW], I16, tag=f"jw128{nm}", name="jw128")
            for g in range(8):
                nc.sync.dma_start(out=jw128[16 * g : 16 * g + 16, :], in_=jw16)
            return jw128

        def gathered_level(c, idx128, wtab, n_elems, bias_off, nm):
            # returns z-rep fp16 [128, GC] (replicated over partitions)
            cs = slice(GC * c, GC * (c + 1))
            g = [
                gpool.tile([128, GC], F32, tag=f"g{k}", name=f"g{k}")
                for k in range(2)
            ]
            for k in range(2):
                nc.gpsimd.ap_gather(
                    out_ap=g[k],
                    in_ap=wtab[k],
                    idxs_ap=idx128,
                    channels=128,
                    num_elems=n_elems,
                    d=1,
                    num_idxs=GC,
                )
            m = [
                gpool.tile([128, GC], F32, tag=f"m{k}", name=f"m{k}")
                for k in range(2)
            ]
            for k in range(2):
                nc.vector.tensor_tensor(
                    out=m[k], in0=x[k][:, cs], in1=g[k], op=mybir.AluOpType.mult
                )
            msum = gpool.tile([128, GC], F32, tag="ms", name="msum")
            nc.vector.tensor_tensor(
                out=msum, in0=m[0], in1=m[1], op=mybir.AluOpType.add
            )
            if use_bias:
                bg = gpool.tile([128, GC], F32, tag="bg", name="bg")
                nc.gpsimd.ap_gather(
                    out_ap=bg,
                    in_ap=b89[:, bias_off : bias_off + n_elems],
                    idxs_ap=idx128,
                    channels=128,
                    num_elems=n_elems,
                    d=1,
                    num_idxs=GC,
                )
                nc.vector.tensor_tensor(
                    out=msum, in0=msum, in1=bg, op=mybir.AluOpType.add
                )
            zrep = gpool.tile([128, GC], F32, tag="zr", name="zrep")
            nc.gpsimd.partition_all_reduce(
                out_ap=zrep,
                in_ap=msum,
                channels=128,
                reduce_op=bass_isa.ReduceOp.add,
            )
            return zrep

        def rep_to_sp(zrep, dst_sp, c, nm):
            # transpose replicated [128, GC] fp16 -> sample-part [128, NTC]
            # 4 blocks of [128, 128] per PSUM bank; evict column 0 of each.
            for q in range(NTC // 4):
                pt = tpool.tile([128, 4, 128], F32, tag="t", name="pt")
                for b in range(4):
                    blk = 4 * q + b
                    nc.tensor.transpose(
                        pt[:, b, :],
                        zrep[:, 128 * blk : 128 * (blk + 1)],
                        ident,
                    )
                nc.scalar.copy(
                    out=dst_sp[:, c * NTC + 4 * q : c * NTC + 4 * q + 4],
                    in_=pt[:, :, 0],
                )

        def jn_from_z(zsp, jprev_slc, jn_out_slc):
            # jn = 2*jprev + (z < 0)
            j2 = ipool.tile([128, NTC], FP16, tag="j2", name="j2")
            nc.vector.tensor_scalar(
                out=j2,
                in0=jprev_slc,
                scalar1=2.0,
                scalar2=None,
                op0=mybir.AluOpType.mult,
            )
            nc.vector.scalar_tensor_tensor(
                out=jn_out_slc,
                in0=zsp,
                scalar=0.0,
                in1=j2,
                op0=mybir.AluOpType.is_lt,
                op1=mybir.AluOpType.add,
            )

        def gathered_chunk(c):
            ji = ipool.tile([128, NTC], I16, tag="j8i", name="ji")
            csl = slice(NTC * c, NTC * (c + 1))
            nc.vector.tensor_copy(out=ji, in_=j8sp[:, csl])
            idx8 = wrap_and_bcast(ji, c, "a")
            z8 = gathered_level(c, idx8, w8, 256, 0, "a")
            z8sp = ipool.tile([128, NTC], FP16, tag="z8sp", name="z8sp")
            rep_to_sp(z8, z8sp, 0, "a")
            jn_from_z(z8sp, j8sp[:, csl], j9sp[:, csl])
            ji9 = ipool.tile([128, NTC], I16, tag="j9i", name="ji9")
            nc.vector.tensor_copy(out=ji9, in_=j9sp[:, csl])
            idx9 = wrap_and_bcast(ji9, c, "b")
            z9 = gathered_level(c, idx9, w9, 512, 256, "b")
            z9sp = ipool.tile([128, NTC], FP16, tag="z9sp", name="z9sp")
            rep_to_sp(z9, z9sp, 0, "b")
            jn_from_z(z9sp, j9sp[:, csl], leafsp[:, csl])
            nc.vector.tensor_copy(out=leafi[:, csl], in_=leafsp[:, csl])

            # final table gathers + output
            for tbl, dview in ((t_out, o_view), (t_std, s_view)):
                rb = opool.tile([128, 2, 128], F32, tag="rb", name="rb")
                nc.gpsimd.ap_gather(
                    out_ap=rb,
                    in_ap=tbl,
                    idxs_ap=leafi[:, csl],
                    channels=128,
                    num_elems=1024,
                    d=1,
                    num_idxs=2 * 128,
                )
                for h in range(2):
                    pt = tpool.tile([128, 128], F32, tag="to", name="pt_o")
                    nc.tensor.transpose(pt, rb[:, h, :], ident)
                    ob = opool.tile([128, 128], F32, tag="ob", name="ob")
                    nc.scalar.copy(out=ob, in_=pt)
                    for qq in range(8):
                        nc.sync.dma_start(
                            out=dview[2 * c + h, qq],
                            in_=ob[16 * qq : 16 * qq + 16, :],
                        )

        # ---------------- main schedule ----------------
        # dense chunks feed gathered chunks (gathered chunk c needs dense
        # chunks 2c, 2c+1); lag the gathered emission to keep PE busy.
        for c in range(NCH):
            dense_chunk(c)
            if c >= 4:
                gathered_chunk(c - 4)

    nc.compile()
    return nc


_CACHE = {}


def _get_nc(use_bias: bool):
    if use_bias not in _CACHE:
        nc = bacc.Bacc("TRN2", target_bir_lowering=False, debug=False)
        _CACHE[use_bias] = _build(nc, use_bias)
    return _CACHE[use_bias]


def _prepare(x, W_pred, b_pred, W_or, action_stds):
    x = np.ascontiguousarray(x, dtype=np.float32)
    W_pred = np.asarray(W_pred, dtype=np.float32)
    b_pred = np.asarray(b_pred, dtype=np.float32)
    W_or = np.asarray(W_or, dtype=np.float32)
    action_stds = np.asarray(action_stds, dtype=np.float32)

    Wd = np.zeros((IN_DIM, DNODES), np.float32)
    Wd[:, :255] = W_pred[:255].T
    W8 = np.ascontiguousarray(W_pred[255:511].T)
    W9 = np.ascontiguousarray(W_pred[511:1023].T)
    # class tables: softmax(W_or) per leaf column, clipped stds; 16 classes
    # replicated 8x on partitions
    m = W_or.max(axis=0, keepdims=True)
    e = np.exp(W_or - m)
    t_out = np.tile((e / e.sum(axis=0, keepdims=True)).astype(np.float32), (8, 1))
    t_std = np.tile(np.clip(action_stds, -20.0, 2.0).astype(np.float32), (8, 1))
    th = np.tile(-b_pred[None, :255], (128, 1)).astype(np.float32)
    thp = np.zeros((128, DNODES), np.float32)
    thp[:, :255] = th
    b89 = np.tile(
        np.concatenate([b_pred[255:511], b_pred[511:1023]])[None, :], (128, 1)
    ).astype(np.float32)
    return x, Wd, W8, W9, t_out, t_std, thp, b89, bool(np.any(b_pred != 0.0))


def kernel(x, W_pred, b_pred, W_or, action_stds, _want_trace=False):
    x, Wd, W8, W9, t_out, t_std, th, b89, use_bias = _prepare(
        x, W_pred, b_pred, W_or, action_stds
    )
    nc = _get_nc(use_bias)

    in_maps = []
    for c in range(N_CORES):
        shard = x[c * B_LOC : (c + 1) * B_LOC]
        in_maps.append(
            {
                "xT": np.ascontiguousarray(shard.T),
                "Wd": Wd,
                "W8": W8,
                "W9": W9,
                "Tout": t_out,
                "Tstd": t_std,
                "TH": th,
                "B89": b89,
                "Ident": np.eye(128, dtype=np.float32),
            }
        )

    res = run_bass_kernel_spmd(
        nc, in_maps, core_ids=list(range(N_CORES)), trace=_want_trace
    )
    out = np.concatenate([res.results[c]["out_o"] for c in range(N_CORES)], axis=0)
    std = np.concatenate([res.results[c]["out_s"] for c in range(N_CORES)], axis=0)
    if _want_trace:
        kernel.last_results = res
    return out, std


# revision 20
# speedup vs baseline: 1.0464x; 1.0464x over previous
"""Trainium2 Bass kernel for nn_DGT_6485400616966 (soft decision tree forward).

Math (forward pass only): the straight-through/one-hot structure collapses to a
10-level tree descent following sign(x . W_pred[node]) at visited nodes;
out = softmax(W_or[:, leaf]); std = clip(action_stds[:, leaf], -20, 2).

Device algorithm per core (8192 samples, data-parallel over 8 cores):
  1. Dense fp32 matmul for tree levels 0..7 only (255 nodes, 256 cols):
     z = x @ W[0:255].T exactly (fp32 PE mode), ACT evicts sign bits.
  2. DVE bottom-up collapse of the 8-level subtree -> j8 [128, 64] (the
     level-8 node index per sample, sample-part layout; sample j sits at
     partition j%128, btile j//128).
  3. Levels 8 and 9 are resolved per sample: wrap j into the gpsimd idx
     layout ([16, Q] wrapped + replicated via small SBUF-SBUF DMAs), gather
     W columns with ap_gather (dim-major, tables resident in SBUF), multiply
     with the same xT tiles on DVE, reduce over the 256 dims with gpsimd
     partition_all_reduce (fp16 out), PE-transpose the replicated result
     back to sample-part, and update j9 = 2*j8 + (z8 < 0), leaf = 2*j9 + u9.
  4. Final ap_gather from class-replicated softmax/std tables by leaf,
     PE transpose, contiguous DMA out in natural sample order.
"""

import sys

for _p in ("/opt/trn_rl_repo",):
    if _p not in sys.path:
        sys.path.insert(0, _p)

from contextlib import ExitStack

import numpy as np

import concourse.bacc as bacc
import concourse.bass as bass
import concourse.bass_isa as bass_isa
import concourse.tile as tile
from concourse import mybir
from concourse.bass_utils import run_bass_kernel_spmd

HEIGHT = 10
IN_DIM = 256
OUT_DIM = 16
BATCH = 65536
N_CORES = 8
B_LOC = BATCH // N_CORES          # 8192 samples per core
NT = B_LOC // 128                 # 64 btiles of 128 samples
NB = 8                            # btiles per dense chunk
NCH = NT // NB                    # 8 dense chunks
DNODES = 256                      # dense cols: nodes 0..254 + 1 pad
GC = 2048                         # samples per gathered-stage chunk
NGC = B_LOC // GC                 # 4 gathered chunks
F32 = mybir.dt.float32
FP16 = mybir.dt.float16
I16 = mybir.dt.int16


def _build(nc, use_bias: bool):
    xT = nc.dram_tensor("xT", [IN_DIM, B_LOC], F32, kind="ExternalInput")
    Wd = nc.dram_tensor("Wd", [IN_DIM, DNODES], F32, kind="ExternalInput")
    W8 = nc.dram_tensor("W8", [IN_DIM, 256], F32, kind="ExternalInput")
    W9 = nc.dram_tensor("W9", [IN_DIM, 512], F32, kind="ExternalInput")
    Tout = nc.dram_tensor("Tout", [128, 1024], F32, kind="ExternalInput")
    Tstd = nc.dram_tensor("Tstd", [128, 1024], F32, kind="ExternalInput")
    TH = nc.dram_tensor("TH", [128, DNODES], F32, kind="ExternalInput")
    B89 = nc.dram_tensor("B89", [128, 768], F32, kind="ExternalInput")
    Ident = nc.dram_tensor("Ident", [128, 128], F32, kind="ExternalInput")
    out_o = nc.dram_tensor("out_o", [B_LOC, OUT_DIM], F32, kind="ExternalOutput")
    out_s = nc.dram_tensor("out_s", [B_LOC, OUT_DIM], F32, kind="ExternalOutput")

    with tile.TileContext(nc) as tc, ExitStack() as ctx:
        consts = ctx.enter_context(tc.tile_pool(name="consts", bufs=1))
        spool = ctx.enter_context(tc.tile_pool(name="spool", bufs=3))
        rpool = ctx.enter_context(tc.tile_pool(name="rpool", bufs=3))
        dpool = ctx.enter_context(tc.tile_pool(name="dpool", bufs=3))
        gpool = ctx.enter_context(tc.tile_pool(name="gpool", bufs=2))
        ipool = ctx.enter_context(tc.tile_pool(name="ipool", bufs=2))
        opool = ctx.enter_context(tc.tile_pool(name="opool", bufs=2))
        zpool = ctx.enter_context(
            tc.tile_pool(name="zpool", bufs=3, space=bass.MemorySpace.PSUM)
        )
        tpool = ctx.enter_context(
            tc.tile_pool(name="tpool", bufs=2, space=bass.MemorySpace.PSUM)
        )

        # persistent tiles
        x = [
            consts.tile([128, B_LOC], F32, tag=f"x{k}", name=f"x{k}")
            for k in range(2)
        ]
        wd = [
            consts.tile([128, DNODES], F32, tag=f"wd{k}", name=f"wd{k}")
            for k in range(2)
        ]
        w8 = [
            consts.tile([128, 256], F32, tag=f"w8{k}", name=f"w8{k}")
            for k in range(2)
        ]
        w9 = [
            consts.tile([128, 512], F32, tag=f"w9{k}", name=f"w9{k}")
            for k in range(2)
        ]
        t_out = consts.tile([128, 1024], F32)
        t_std = consts.tile([128, 1024], F32)
        ident = consts.tile([128, 128], F32)
        ident16 = consts.tile([128, 128], FP16)
        th = consts.tile([128, DNODES], F32) if use_bias else None
        b89 = consts.tile([128, 768], F32) if use_bias else None

        j8sp = consts.tile([128, NT], FP16)   # level-8 node idx, sample-part
        j9sp = consts.tile([128, NT], FP16)
        leafsp = consts.tile([128, NT], FP16)
        leafi = consts.tile([128, NT], I16)

        # late-loaded consts (first consumed after the first dense chunk)
        def load_late_consts():
            for k in range(2):
                ks = slice(128 * k, 128 * (k + 1))
                nc.sync.dma_start(out=wd[k], in_=Wd[ks, :])
                nc.sync.dma_start(out=w8[k], in_=W8[ks, :])
                nc.sync.dma_start(out=w9[k], in_=W9[ks, :])
            nc.sync.dma_start(out=t_out, in_=Tout[:, :])
            nc.sync.dma_start(out=t_std, in_=Tstd[:, :])
            nc.sync.dma_start(out=ident, in_=Ident[:, :])
            nc.vector.tensor_copy(out=ident16, in_=ident)
            if use_bias:
                nc.sync.dma_start(out=th, in_=TH[:, :])
                nc.sync.dma_start(out=b89, in_=B89[:, :])

        # W for dense chunk 0 must be there first
        nc.sync.dma_start(out=wd[0][:, :], in_=Wd[0:128, :])
        nc.sync.dma_start(out=wd[1][:, :], in_=Wd[128:256, :])

        # device writes blocks in transpose-native order; host unscrambles
        o_view = out_o.rearrange("(b i e) c -> b i (e c)", b=8, i=128, e=8)
        s_view = out_s.rearrange("(b i e) c -> b i (e c)", b=8, i=128, e=8)

        # ---------------- dense stage: one chunk of NB btiles ----------------
        def dense_chunk(c):
            s_chunk = spool.tile([128, NB, DNODES], FP16, tag="s")
            for k in range(NB):
                t = c * NB + k
                if k == 0:
                    hs = slice(128 * NB * c, 128 * NB * (c + 1))
                    for kk in range(2):
                        ks = slice(128 * kk, 128 * (kk + 1))
                        nc.sync.dma_start(out=x[kk][:, hs], in_=xT[ks, hs])
                    if c == 0:
                        load_late_consts()
                bs = slice(128 * t, 128 * (t + 1))
                z = zpool.tile([128, DNODES], F32, tag="z", name="z")
                for kk in range(2):
                    nc.tensor.matmul(
                        z[:, :],
                        x[kk][:, bs],
                        wd[kk][:, :],
                        start=(kk == 0),
                        stop=(kk == 1),
                    )
                # u = (z < -b): ACT saturated-sigmoid trick for b == 0
                if use_bias:
                    nc.vector.tensor_tensor(
                        out=s_chunk[:, k, :],
                        in0=z[:, :],
                        in1=th[:, :],
                        op=mybir.AluOpType.is_lt,
                    )
                else:
                    nc.scalar.activation(
                        out=s_chunk[:, k, :],
                        in_=z[:, :],
                        func=mybir.ActivationFunctionType.Sigmoid,
                        scale=-1e30,
                    )

            # bottom-up collapse of the 8-level subtree (nodes 0..254)
            r_prev = s_chunk[:, :, 127:255]
            for i in range(6, -1, -1):
                n = 1 << i
                kconst = float(1 << (7 - i))
                u_i = s_chunk[:, :, n - 1 : 2 * n - 1]
                rp = r_prev.rearrange("p b (n two) -> p b n two", two=2)
                r_e = rp[:, :, :, 0]
                r_o = rp[:, :, :, 1]
                d_t = dpool.tile([128, NB, n], FP16, tag="d", name="d_t")
                nc.vector.scalar_tensor_tensor(
                    out=d_t,
                    in0=r_o,
                    scalar=kconst,
                    in1=r_e,
                    op0=mybir.AluOpType.add,
                    op1=mybir.AluOpType.subtract,
                )
                nc.vector.tensor_tensor(
                    out=d_t, in0=u_i, in1=d_t, op=mybir.AluOpType.mult
                )
                if i > 0:
                    r_t = rpool.tile([128, NB, n], FP16, tag="r", name="r_t")
                    nc.vector.tensor_tensor(
                        out=r_t, in0=r_e, in1=d_t, op=mybir.AluOpType.add
                    )
                    r_prev = r_t
                else:
                    nc.vector.tensor_tensor(
                        out=j8sp[:, c * NB : (c + 1) * NB],
                        in0=r_e[:, :, 0],
                        in1=d_t[:, :, 0],
                        op=mybir.AluOpType.add,
                    )

        # ------------- gathered-level helpers (per chunk of GC) -------------
        QW = GC // 16            # wrapped free size (128)
        NTC = GC // 128          # btiles per gathered chunk (16)

        def wrap_and_bcast(src_tile, csl, slot, eng, tagn):
            # [128, NTC] fp16 sample-part -> int16 -> wrapped into partitions
            # 0..15, then replicated to all 8 groups by 3 doubling DMAs.
            ji = ipool.tile([128, NTC], I16, tag=f"ji{tagn}", name="ji")
            nc.vector.tensor_copy(out=ji, in_=src_tile[:, csl])
            jw128 = ipool.tile([128, QW], I16, tag=f"jw{tagn}", name="jw128")
            wv = jw128.rearrange("p (b k) -> p k b", k=8)
            for k in range(8):
                eng.dma_start(
                    out=wv[0:16, k, :],
                    in_=ji[16 * k : 16 * k + 16, :],
                )
            for d in (16, 32, 64):
                eng.dma_start(out=jw128[d : 2 * d, :], in_=jw128[0:d, :])
            return jw128

        def gathered_level(c, idx128, wtab, n_elems, bias_off, nm):
            # returns z-rep fp16 [128, GC] (replicated over partitions)
            cs = slice(GC * c, GC * (c + 1))
            g = [
                gpool.tile([128, GC], F32, tag=f"g{k}", name=f"g{k}")
                for k in range(2)
            ]
            for k in range(2):
                nc.gpsimd.ap_gather(
                    out_ap=g[k],
                    in_ap=wtab[k],
                    idxs_ap=idx128,
                    channels=128,
                    num_elems=n_elems,
                    d=1,
                    num_idxs=GC,
                )
            m = [
                gpool.tile([128, GC], F32, tag=f"m{k}", name=f"m{k}")
                for k in range(2)
            ]
            for k in range(2):
                nc.vector.tensor_tensor(
                    out=m[k], in0=x[k][:, cs], in1=g[k], op=mybir.AluOpType.mult
                )
            msum = gpool.tile([128, GC], F32, tag="ms", name="msum")
            nc.vector.tensor_tensor(
                out=msum, in0=m[0], in1=m[1], op=mybir.AluOpType.add
            )
            if use_bias:
                bg = gpool.tile([128, GC], F32, tag="bg", name="bg")
                nc.gpsimd.ap_gather(
                    out_ap=bg,
                    in_ap=b89[:, bias_off : bias_off + n_elems],
                    idxs_ap=idx128,
                    channels=128,
                    num_elems=n_elems,
                    d=1,
                    num_idxs=GC,
                )
                nc.vector.tensor_tensor(
                    out=msum, in0=msum, in1=bg, op=mybir.AluOpType.add
                )
            zrep = gpool.tile([128, GC], F32, tag="zr", name="zrep")
            nc.gpsimd.partition_all_reduce(
                out_ap=zrep,
                in_ap=msum,
                channels=128,
                reduce_op=bass_isa.ReduceOp.add,
            )
            return zrep

        def rep_to_sp(zrep, dst_sp, c, nm):
            # transpose replicated [128, GC] fp16 -> sample-part [128, NTC]
            # 4 blocks of [128, 128] per PSUM bank; evict column 0 of each.
            for q in range(NTC // 4):
                pt = tpool.tile([128, 4, 128], F32, tag="t", name="pt")
                for b in range(4):
                    blk = 4 * q + b
                    nc.tensor.transpose(
                        pt[:, b, :],
                        zrep[:, 128 * blk : 128 * (blk + 1)],
                        ident,
                    )
                nc.scalar.copy(
                    out=dst_sp[:, c * NTC + 4 * q : c * NTC + 4 * q + 4],
                    in_=pt[:, :, 0],
                )

        def jn_from_z(zsp, jprev_slc, jn_out_slc):
            # jn = 2*jprev + (z < 0)
            j2 = ipool.tile([128, NTC], FP16, tag="j2", name="j2")
            nc.vector.tensor_scalar(
                out=j2,
                in0=jprev_slc,
                scalar1=2.0,
                scalar2=None,
                op0=mybir.AluOpType.mult,
            )
            nc.vector.scalar_tensor_tensor(
                out=jn_out_slc,
                in0=zsp,
                scalar=0.0,
                in1=j2,
                op0=mybir.AluOpType.is_lt,
                op1=mybir.AluOpType.add,
            )

        def wrap_j8(c):
            csl = slice(NTC * c, NTC * (c + 1))
            return wrap_and_bcast(j8sp, csl, (c % 2) * 2, nc.sync, "a")

        def gathered_chunk(c, idx8):
            csl = slice(NTC * c, NTC * (c + 1))
            z8 = gathered_level(c, idx8, w8, 256, 0, "a")
            z8sp = ipool.tile([128, NTC], FP16, tag="z8sp", name="z8sp")
            rep_to_sp(z8, z8sp, 0, "a")
            jn_from_z(z8sp, j8sp[:, csl], j9sp[:, csl])
            idx9 = wrap_and_bcast(j9sp, csl, (c % 2) * 2 + 1, nc.scalar, "b")
            z9 = gathered_level(c, idx9, w9, 512, 256, "b")
            z9sp = ipool.tile([128, NTC], FP16, tag="z9sp", name="z9sp")
            rep_to_sp(z9, z9sp, 0, "b")
            jn_from_z(z9sp, j9sp[:, csl], leafsp[:, csl])
            nc.vector.tensor_copy(out=leafi[:, csl], in_=leafsp[:, csl])

            # final table gathers + output
            for tbl, dview in ((t_out, o_view), (t_std, s_view)):
                rb = opool.tile([128, 2, 128], F32, tag="rb", name="rb")
                nc.gpsimd.ap_gather(
                    out_ap=rb,
                    in_ap=tbl,
                    idxs_ap=leafi[:, csl],
                    channels=128,
                    num_elems=1024,
                    d=1,
                    num_idxs=2 * 128,
                )
                for h in range(2):
                    pt = tpool.tile([128, 128], F32, tag="to", name="pt_o")
                    nc.tensor.transpose(pt, rb[:, h, :], ident)
                    ob = opool.tile([128, 128], F32, tag="ob", name="ob")
                    nc.scalar.copy(out=ob, in_=pt)
                    nc.sync.dma_start(out=dview[2 * c + h], in_=ob)

        # ---------------- main schedule ----------------
        # dense chunks feed gathered chunks (gathered chunk c needs dense
        # chunks 2c, 2c+1); lag the gathered emission to keep PE busy.
        idx8_q = {}
        for c in range(NCH):
            dense_chunk(c)
            if c % 2 == 1:
                idx8_q[c // 2] = wrap_j8(c // 2)
            if c >= 4:
                gathered_chunk(c - 4, idx8_q.pop(c - 4))

    nc.compile()
    return nc


# device block layout: o_dev[b, i, g, :] = OUT[sample 1024b + 128*(i//16) + 16g + i%16]
def _out_perm():
    b, i, g = np.meshgrid(
        np.arange(8), np.arange(128), np.arange(8), indexing="ij"
    )
    return (1024 * b + 128 * (i // 16) + 16 * g + i % 16).ravel()


_PERM = _out_perm()


def _unscramble(dev):
    out = np.empty((B_LOC, OUT_DIM), np.float32)
    out[_PERM] = dev.reshape(-1, OUT_DIM)
    return out


_CACHE = {}


def _get_nc(use_bias: bool):
    if use_bias not in _CACHE:
        nc = bacc.Bacc("TRN2", target_bir_lowering=False, debug=False)
        _CACHE[use_bias] = _build(nc, use_bias)
    return _CACHE[use_bias]


def _prepare(x, W_pred, b_pred, W_or, action_stds):
    x = np.ascontiguousarray(x, dtype=np.float32)
    W_pred = np.asarray(W_pred, dtype=np.float32)
    b_pred = np.asarray(b_pred, dtype=np.float32)
    W_or = np.asarray(W_or, dtype=np.float32)
    action_stds = np.asarray(action_stds, dtype=np.float32)

    Wd = np.zeros((IN_DIM, DNODES), np.float32)
    Wd[:, :255] = W_pred[:255].T
    W8 = np.ascontiguousarray(W_pred[255:511].T)
    W9 = np.ascontiguousarray(W_pred[511:1023].T)
    # class tables: softmax(W_or) per leaf column, clipped stds; 16 classes
    # replicated 8x on partitions
    m = W_or.max(axis=0, keepdims=True)
    e = np.exp(W_or - m)
    t_out = np.tile((e / e.sum(axis=0, keepdims=True)).astype(np.float32), (8, 1))
    t_std = np.tile(np.clip(action_stds, -20.0, 2.0).astype(np.float32), (8, 1))
    th = np.tile(-b_pred[None, :255], (128, 1)).astype(np.float32)
    thp = np.zeros((128, DNODES), np.float32)
    thp[:, :255] = th
    b89 = np.tile(
        np.concatenate([b_pred[255:511], b_pred[511:1023]])[None, :], (128, 1)
    ).astype(np.float32)
    return x, Wd, W8, W9, t_out, t_std, thp, b89, bool(np.any(b_pred != 0.0))


def kernel(x, W_pred, b_pred, W_or, action_stds, _want_trace=False):
    x, Wd, W8, W9, t_out, t_std, th, b89, use_bias = _prepare(
        x, W_pred, b_pred, W_or, action_stds
    )
    nc = _get_nc(use_bias)

    in_maps = []
    for c in range(N_CORES):
        shard = x[c * B_LOC : (c + 1) * B_LOC]
        in_maps.append(
            {
                "xT": np.ascontiguousarray(shard.T),
                "Wd": Wd,
                "W8": W8,
                "W9": W9,
                "Tout": t_out,
                "Tstd": t_std,
                "TH": th,
                "B89": b89,
                "Ident": np.eye(128, dtype=np.float32),
            }
        )

    res = run_bass_kernel_spmd(
        nc, in_maps, core_ids=list(range(N_CORES)), trace=_want_trace
    )
    out = np.concatenate(
        [_unscramble(res.results[c]["out_o"]) for c in range(N_CORES)], axis=0
    )
    std = np.concatenate(
        [_unscramble(res.results[c]["out_s"]) for c in range(N_CORES)], axis=0
    )
    if _want_trace:
        kernel.last_results = res
    return out, std


# revision 21
# speedup vs baseline: 5.8714x; 5.6109x over previous
"""Trainium2 Bass kernel for nn_DGT_6485400616966 (soft decision tree forward).

Math (forward pass only):
  pred_z = x @ W_pred.T + b_pred                      [B, 1023]
  The straight-through/one-hot structure collapses: the output depends only on
  the argmax leaf of the tree AND layer, which equals a 10-level tree descent
  following sign(pred_z) at visited nodes (left if z >= 0).
  out = softmax(W_or[:, leaf]) ; std = clip(action_stds[:, leaf], -20, 2)

Device algorithm per core (8192 samples, data-parallel over 8 cores):
  1. PE: z = x @ W_pred.T in three fp32r passes (xh@wh + xh@wl + xl@wh) where
     hi/lo are an exact e8m11 split of the fp32 operands (fp32r on HW is
     e8m11; one pass alone flips ~38 argmax rows, three passes flip none).
     x tiles are the stationary operand; W^T columns (nodes, padded to 1024)
     are the moving operand, N=512 per matmul for full fp32r rate.
  2. Eviction PSUM->SBUF per btile: u = (z < 0) as fp16, contiguous writes
     (strided 2-byte DVE writes cost ~4x). Split DVE tensor_scalar is_lt /
     ACT saturated-sigmoid (Sigmoid(-1e30*z) is exactly {0,1}).
  3. DVE: bottom-up tree collapse r_i = r_e + u_i*(K + r_o - r_e) in fp16 on
     [128, NB, 2^i] chunk tensors (btile-major; all writes contiguous).
  4. GPSIMD ap_gather per chunk: table lookup T[class, leaf] with the 16
     classes replicated on partitions; each 16-partition group shares its
     sample's leaf index (host pre-permutes rows by pi(p)=8*(p%16)+p//16 so
     indices are already wrapped and outputs land in natural order).
  5. PE transpose of the gathered [128, 128] blocks (emitted LAG chunks late
     so the in-order PE queue never stalls) + contiguous DMA out.
"""

import sys

for _p in ("/opt/trn_rl_repo",):
    if _p not in sys.path:
        sys.path.insert(0, _p)

from contextlib import ExitStack

import numpy as np

import concourse.bacc as bacc
import concourse.bass as bass
import concourse.tile as tile
from concourse import mybir
from concourse.bass_utils import run_bass_kernel_spmd

HEIGHT = 10
IN_DIM = 256
OUT_DIM = 16
BATCH = 65536
N_CORES = 8
B_LOC = BATCH // N_CORES          # 8192 samples per core
NT = B_LOC // 128                 # 64 batch tiles of 128 samples
NB = 8                            # btiles per collapse chunk
NCH = NT // NB                    # 4 chunks
NODES = 1024                      # 1023 real + 1 pad
F32 = mybir.dt.float32
F32R = mybir.dt.float32r
BF16 = mybir.dt.bfloat16
FP16 = mybir.dt.float16
I16 = mybir.dt.int16


def _build(nc, use_sign_path: bool):
    """Emit the per-core program. use_sign_path=True assumes b_pred == 0."""
    # hi/lo e8m11 split operands (fp32r is e8m11 on HW; hi+lo == fp32 exactly)
    xTh = nc.dram_tensor("xTh", [IN_DIM, B_LOC], F32R, kind="ExternalInput")
    xTl = nc.dram_tensor("xTl", [IN_DIM, B_LOC], BF16, kind="ExternalInput")
    Wph = nc.dram_tensor("Wph", [IN_DIM, NODES], F32R, kind="ExternalInput")
    Wpl = nc.dram_tensor("Wpl", [IN_DIM, NODES], F32R, kind="ExternalInput")
    Wpb = nc.dram_tensor("Wpb", [IN_DIM, NODES], BF16, kind="ExternalInput")
    Tout = nc.dram_tensor("Tout", [128, NODES], F32, kind="ExternalInput")
    Tstd = nc.dram_tensor("Tstd", [128, NODES], F32, kind="ExternalInput")
    TH = nc.dram_tensor("TH", [128, NODES], F32, kind="ExternalInput")
    Ident = nc.dram_tensor("Ident", [128, 128], F32, kind="ExternalInput")
    out_o = nc.dram_tensor("out_o", [B_LOC, OUT_DIM], F32, kind="ExternalOutput")
    out_s = nc.dram_tensor("out_s", [B_LOC, OUT_DIM], F32, kind="ExternalOutput")

    with tile.TileContext(nc) as tc, ExitStack() as ctx:
        consts = ctx.enter_context(tc.tile_pool(name="consts", bufs=1))
        xpool = ctx.enter_context(tc.tile_pool(name="xpool", bufs=4))
        spool = ctx.enter_context(tc.tile_pool(name="spool", bufs=3))
        rpool = ctx.enter_context(tc.tile_pool(name="rpool", bufs=3))
        dpool = ctx.enter_context(tc.tile_pool(name="dpool", bufs=3))
        zpool = ctx.enter_context(
            tc.tile_pool(name="zpool", bufs=3, space=bass.MemorySpace.PSUM)
        )
        tpool = ctx.enter_context(
            tc.tile_pool(name="tpool", bufs=2, space=bass.MemorySpace.PSUM)
        )

        wh = [
            consts.tile([128, NODES], F32R, tag=f"wh{k}", name=f"wh{k}")
            for k in range(2)
        ]
        wl = [
            consts.tile([128, NODES], F32R, tag=f"wl{k}", name=f"wl{k}")
            for k in range(2)
        ]
        whb = [
            consts.tile([128, NODES], BF16, tag=f"whb{k}", name=f"whb{k}")
            for k in range(2)
        ]
        nc.sync.dma_start(out=wh[0], in_=Wph[0:128, :])
        nc.sync.dma_start(out=whb[0], in_=Wpb[0:128, :])

        def load_late_weights():
            nc.sync.dma_start(out=wl[0], in_=Wpl[0:128, :])
            nc.sync.dma_start(out=wh[1], in_=Wph[128:256, :])
            nc.sync.dma_start(out=wl[1], in_=Wpl[128:256, :])
            nc.sync.dma_start(out=whb[1], in_=Wpb[128:256, :])
        t_out = consts.tile([128, NODES], F32)
        t_std = consts.tile([128, NODES], F32)
        ident = consts.tile([128, 128], F32)
        th = None
        if not use_sign_path:
            th = consts.tile([128, NODES], F32)
            nc.sync.dma_start(out=th, in_=TH[:, :])

        def load_late_consts():
            # tables/identity are first consumed by the descent/output stage;
            # loading them after the first chunk's x keeps the PE start early.
            nc.sync.dma_start(out=t_out, in_=Tout[:, :])
            nc.sync.dma_start(out=t_std, in_=Tstd[:, :])
            nc.sync.dma_start(out=ident, in_=Ident[:, :])

        leaf_all = consts.tile([128, NT], FP16)
        leaf_i16 = consts.tile([128, NT], I16)
        r_out = consts.tile([128, NODES], F32)
        r_std = consts.tile([128, NODES], F32)

        o_view = out_o.rearrange("(t p f) c -> t p (f c)", t=8, p=128, f=8)
        s_view = out_s.rearrange("(t p f) c -> t p (f c)", t=8, p=128, f=8)
        LAG = 3

        def emit_out_chain(cc):
            # transpose chunk cc's gathered [128, 128] table blocks and DMA
            # them out; emitted LAG chunks late so the in-order PE queue
            # never stalls on the descent chain.
            rs_ = slice(128 * cc, 128 * (cc + 1))
            for rbuf, dview in ((r_out, o_view), (r_std, s_view)):
                pt = tpool.tile([128, 128], F32, tag="t", name="pt")
                nc.tensor.transpose(pt, rbuf[:, rs_], ident)
                rt = xpool.tile([128, 128], F32, tag="rt", name="rt", bufs=2)
                nc.scalar.copy(out=rt, in_=pt)
                nc.sync.dma_start(out=dview[cc], in_=rt)

        for c in range(NCH):
            # btile-MAJOR u-bit store: eviction writes [128, 1024] contiguous
            # (strided 2-byte writes cost ~4x on DVE; reads don't).
            s_chunk = spool.tile([128, NB, NODES], FP16, tag="s")
            for k in range(NB):
                t = c * NB + k
                bs = slice(128 * t, 128 * (t + 1))
                if k == 0:
                    # stage x for this chunk: [128, 128*NB] per ktile/half
                    hs = slice(128 * NB * c, 128 * NB * (c + 1))
                    xh = [
                        xpool.tile(
                            [128, 128 * NB], F32R,
                            tag=f"xh{kk}", name=f"xh{kk}", bufs=2,
                        )
                        for kk in range(2)
                    ]
                    xl = [
                        xpool.tile(
                            [128, 128 * NB], BF16,
                            tag=f"xl{kk}", name=f"xl{kk}", bufs=2,
                        )
                        for kk in range(2)
                    ]
                    for kk in range(2):
                        ks = slice(128 * kk, 128 * (kk + 1))
                        nc.sync.dma_start(out=xh[kk], in_=xTh[ks, hs])
                        nc.sync.dma_start(out=xl[kk], in_=xTl[ks, hs])
                    if c == 0:
                        load_late_weights()
                        load_late_consts()
                kb = slice(128 * k, 128 * (k + 1))
                z = zpool.tile([128, NODES], F32, tag="z")
                # z = xh@wh + xh@wl + xl@wh  (xl@wl term negligible)
                pair = 0
                for kk in range(2):
                    for lhs, rhs in (
                        (xh[kk], wh[kk]),
                        (xh[kk], wl[kk]),
                        (xl[kk], whb[kk]),
                    ):
                        for nh in range(2):
                            ns = slice(512 * nh, 512 * (nh + 1))
                            nc.tensor.matmul(
                                z[:, ns],
                                lhs[:, kb],
                                rhs[:, ns],
                                start=(pair == 0),
                                stop=(pair == 5),
                            )
                        pair += 1
                # u = (z < -b_pred); contiguous [128, 1024] write.
                # Explicit DVE/ACT split: ACT eviction uses the saturated
                # sigmoid trick u = Sigmoid(-1e30 * z) which is exactly
                # {0, 1} fp for any |z| > 1e-28.
                if use_sign_path:
                    if k % 8 < 4:
                        nc.scalar.activation(
                            out=s_chunk[:, k, :],
                            in_=z[:, :],
                            func=mybir.ActivationFunctionType.Sigmoid,
                            scale=-1e30,
                        )
                    else:
                        nc.vector.tensor_scalar(
                            out=s_chunk[:, k, :],
                            in0=z[:, :],
                            scalar1=0.0,
                            scalar2=None,
                            op0=mybir.AluOpType.is_lt,
                        )
                else:
                    nc.vector.tensor_tensor(
                        out=s_chunk[:, k, :],
                        in0=z[:, :],
                        in1=th[:, :],
                        op=mybir.AluOpType.is_lt,
                    )

            # ---- bottom-up collapse (fp16; all WRITES contiguous) ----
            # r_9 = u at level-9 nodes (columns 511..1022)
            r_prev = s_chunk[:, :, 511:1023]
            for i in range(8, -1, -1):
                n = 1 << i
                kconst = float(1 << (9 - i))
                u_i = s_chunk[:, :, n - 1 : 2 * n - 1]
                rp = r_prev.rearrange("p b (n two) -> p b n two", two=2)
                r_e = rp[:, :, :, 0]
                r_o = rp[:, :, :, 1]
                d_t = dpool.tile([128, NB, n], FP16, tag="d")
                # D = (r_o + K) - r_e
                nc.vector.scalar_tensor_tensor(
                    out=d_t,
                    in0=r_o,
                    scalar=kconst,
                    in1=r_e,
                    op0=mybir.AluOpType.add,
                    op1=mybir.AluOpType.subtract,
                )
                # D *= u
                nc.vector.tensor_tensor(
                    out=d_t, in0=u_i, in1=d_t, op=mybir.AluOpType.mult
                )
                # r = r_e + D
                if i > 0:
                    r_t = rpool.tile([128, NB, n], FP16, tag="r")
                    nc.vector.tensor_tensor(
                        out=r_t, in0=r_e, in1=d_t, op=mybir.AluOpType.add
                    )
                    r_prev = r_t
                else:
                    nc.vector.tensor_tensor(
                        out=leaf_all[:, c * NB : (c + 1) * NB],
                        in0=r_e[:, :, 0],
                        in1=d_t[:, :, 0],
                        op=mybir.AluOpType.add,
                    )

            # ---- per-chunk output stage ----
            # leaf -> int16. Sample rows are host-permuted within each
            # 128-block by pi(p) = 8*(p%16) + p//16, so leaf_i16 is already
            # in ap_gather's wrapped index layout and outputs land in
            # natural row order.
            cslice = slice(NB * c, NB * (c + 1))
            nc.vector.tensor_copy(
                out=leaf_i16[:, cslice], in_=leaf_all[:, cslice]
            )
            # table gathers: R[16g+cls, j] = T[cls, leaf(sample 8j+g)]
            rs = slice(128 * c, 128 * (c + 1))
            for tbl, rbuf in ((t_out, r_out), (t_std, r_std)):
                nc.gpsimd.ap_gather(
                    out_ap=rbuf[:, rs],
                    in_ap=tbl,
                    idxs_ap=leaf_i16[:, cslice],
                    channels=128,
                    num_elems=NODES,
                    d=1,
                    num_idxs=128,
                )
            if c >= LAG:
                emit_out_chain(c - LAG)

        for c in range(NCH - LAG, NCH):
            emit_out_chain(c)

    nc.compile()
    return nc


_CACHE = {}


def _get_nc(use_sign_path: bool):
    key = use_sign_path
    if key not in _CACHE:
        nc = bacc.Bacc("TRN2", target_bir_lowering=False, debug=False)
        _CACHE[key] = _build(nc, use_sign_path)
    return _CACHE[key]


# Within each 128-row block, device partition p holds sample row PERM[p].
# PERM aligns the collapse output with ap_gather's wrapped index layout and
# makes the final outputs land in natural row order (see kernel() docstring).
PERM = np.array([8 * (p % 16) + p // 16 for p in range(128)], dtype=np.int64)


def _e8m11(x):
    """Round fp32 to the HW fp32r format (8-bit exp, 11-bit mantissa, RNE)."""
    u = np.ascontiguousarray(x, np.float32).view(np.uint32)
    low = u & np.uint32(0xFFF)
    base = u & np.uint32(0xFFFFF000)
    add = (low > 0x800) | ((low == 0x800) & ((u >> 12) & 1).astype(bool))
    return (base + np.where(add, np.uint32(0x1000), np.uint32(0))).view(np.float32)


def _split_hi_lo(a, lo_bf16=False):
    hi = _e8m11(a)
    lo = (a - hi).astype(np.float32)  # exactly e8m11-representable
    if lo_bf16:
        import ml_dtypes
        lo = lo.astype(ml_dtypes.bfloat16)
    return hi, lo


def _shard_xT(x_shard):
    """[8192, 256] sample rows -> permuted, transposed [256, 8192] device input."""
    xp = x_shard.reshape(NT, 128, IN_DIM)[:, PERM, :].reshape(B_LOC, IN_DIM)
    return np.ascontiguousarray(xp.T)


def _prepare(x, W_pred, b_pred, W_or, action_stds):
    x = np.ascontiguousarray(x, dtype=np.float32)
    W_pred = np.asarray(W_pred, dtype=np.float32)
    b_pred = np.asarray(b_pred, dtype=np.float32)
    W_or = np.asarray(W_or, dtype=np.float32)
    action_stds = np.asarray(action_stds, dtype=np.float32)

    n_int = 2**HEIGHT - 1
    Wp = np.zeros((IN_DIM, NODES), np.float32)
    Wp[:, :n_int] = W_pred.T
    Wph, Wpl = _split_hi_lo(Wp)
    import ml_dtypes
    Wpb = Wph.astype(ml_dtypes.bfloat16)
    # softmax over classes per leaf column
    m = W_or.max(axis=0, keepdims=True)
    e = np.exp(W_or - m)
    t_out16 = (e / e.sum(axis=0, keepdims=True)).astype(np.float32)  # [16, 1024]
    t_std16 = np.clip(action_stds, -20.0, 2.0).astype(np.float32)
    t_out = np.tile(t_out16, (8, 1))  # [128, 1024]
    t_std = np.tile(t_std16, (8, 1))
    th16 = np.zeros((NODES,), np.float32)
    th16[:n_int] = -b_pred
    th = np.tile(th16[None, :], (128, 1))
    return x, Wph, Wpl, Wpb, t_out, t_std, th, bool(np.any(b_pred != 0.0))


def kernel(x, W_pred, b_pred, W_or, action_stds, _want_trace=False):
    x, Wph, Wpl, Wpb, t_out, t_std, th, b_nonzero = _prepare(
        x, W_pred, b_pred, W_or, action_stds
    )
    nc = _get_nc(use_sign_path=not b_nonzero)

    in_maps = []
    for c in range(N_CORES):
        shard = x[c * B_LOC : (c + 1) * B_LOC]
        xt = _shard_xT(shard)
        xth, xtl = _split_hi_lo(xt, lo_bf16=True)
        in_maps.append(
            {
                "xTh": xth,
                "xTl": xtl,
                "Wph": Wph,
                "Wpl": Wpl,
                "Wpb": Wpb,
                "Tout": t_out,
                "Tstd": t_std,
                "TH": th,
                "Ident": np.eye(128, dtype=np.float32),
            }
        )

    res = run_bass_kernel_spmd(
        nc, in_maps, core_ids=list(range(N_CORES)), trace=_want_trace
    )
    out = np.concatenate([res.results[c]["out_o"] for c in range(N_CORES)], axis=0)
    std = np.concatenate([res.results[c]["out_s"] for c in range(N_CORES)], axis=0)
    if _want_trace:
        kernel.last_results = res
    return out, std

